# revision 25
# baseline (speedup 1.0000x reference)
"""Trainium2 Bass kernel for nn_LocalAttention_28518582845970.

The reference projects the full 256x256x1024 grid through Q/K/V/O but
returns only out[px, py] -- a single 1024-vector.  That vector depends
on one window row: 129 tokens, one query token, and the four 1024x1024
weights (by linearity, softmax shift-invariance, and sum(attn)==1):

    q      = Wq t_q + bq
    u      = (Wk/32)^T q                 (q.bk const in k -> dropped)
    scores = tokens @ u
    ex     = exp(scores)                 (unnormalized; /sum folded into
                                          the t_avg cast)
    t_avg  = (ex @ tokens) / sum(ex)
    ctx_c  = Wv[S_c,:] t_avg + bv[S_c]   (e-contraction shard, 128/core)
    part_c = Wo[:,S_c] ctx_c (+ bo on core 0)
    out    = sum_c part_c                (host-side unshard of the
                                          sum-sharded output)

v6 vs v5 (39.9us) vs v3 (46.4us):
  * all sub-512B-line inputs (t_q, biases, tail-token columns) ride as a
    25-column fp16 header on the wq stream -- v5's tiny-descriptor DMAs
    (16-36 B lines) were HBM-latency-bound and held DMA-completion lanes
    for ~5us, stalling the weight stream issue.
  * enable_partition_id=False drops the ~1.2us per-engine TENSOR_LOAD
    from the preamble.
  * window token #128 contributes via two DVE ops on pre-packed
    [128, 8] e-major columns instead of 8 K=1 matmuls.
  * exp emits fp16 directly; attn stays unnormalized through the t_avg
    matmul; 1/sum(exp) is folded into the tv cast (saves a DVE pass and
    shortens the critical chain).
  * out-stage: both 512-halves share the ctx stationary then the ones
    stationary (4 pipelined matmuls instead of 4 isolated ones); each
    half is copied (DVE / ACT in parallel) and DMA'd as its own 2 KiB
    single-descriptor row as soon as it is ready.
"""

import os
import sys

os.environ.setdefault("JAX_PLATFORMS", "axon,cpu")

for _p in ("/opt/trn_rl_repo", "/root/.axon_site/_ro/trn_rl_repo"):
    if os.path.isdir(_p) and _p not in sys.path:
        sys.path.append(_p)

import numpy as np

import concourse.bass as bass
import concourse.mybir as mybir
import concourse.tile as tile
from concourse import bacc
from concourse.bass_utils import run_bass_kernel_spmd

N_CORES = 8
E = 1024
EC = E // 128
WIN = 64
H = W = 256
SCALE = 1.0 / 32.0
HDR = 25            # wq header columns: tq(8) bq(8) bv(1) t128cols(8)
F32 = mybir.dt.float32
F16 = mybir.dt.float16

_BUILD_CACHE: dict = {}

from concourse.vector_clock import ScopedClock as _ScopedClock


def _light_drain_and_barrier(self, tick_clock, wait_clock):
    drain_inst = self.nc.sync.drain()
    wait_clock.add_sem_waits(
        drain_inst.ins, _ScopedClock({None: tick_clock.global_clock})
    )
    self.nc.all_engine_barrier(sem_only=True)
    popped = self.nc._tile_sem_poison_stack.pop()
    assert popped is self._sem_poison
    self.nc.clear_and_free_semaphores(list(self.sems.allocated().values()))
    self.nc.all_engine_barrier(sem_only=True)


tile.TileContext._drain_and_barrier = _light_drain_and_barrier


def _build(L: int, qidx: int):
    KA = min(128, L)
    LT = L - KA               # tail tokens (1 for L=129)
    assert LT in (0, 1)

    nc = bacc.Bacc(None, target_bir_lowering=False, debug=False,
                   enable_partition_id=False)

    wq_d = nc.dram_tensor("wqx", [128, HDR + EC * E], F16, kind="ExternalInput")
    wk_d = nc.dram_tensor("wk", [128, EC * E], F16, kind="ExternalInput")
    tokT_d = nc.dram_tensor("tokT", [128, EC * L], F16, kind="ExternalInput")
    jmb_d = nc.dram_tensor("jmb", [128, 2 * E], F16, kind="ExternalInput")
    out_d = nc.dram_tensor("out", [1, E], F16, kind="ExternalOutput")

    with tile.TileContext(nc) as tc:
        with (
            tc.tile_pool(name="consts", bufs=1) as consts,
            tc.tile_pool(name="sbw", bufs=1) as sbw,
            tc.tile_pool(name="psS", bufs=2, space="PSUM") as psS,
        ):
            # ---- SBUF tiles ----
            wq_sb = consts.tile([128, HDR + EC * E], F16)   # hdr | [fc,ec,f]
            wk_sb = consts.tile([128, EC, EC, 128], F16)    # [pf, ec, fc, e]
            tokT_sb = consts.tile([128, EC, L], F16)        # [pe, ec, k]
            jmb_sb = consts.tile([128, 2 * E], F16)         # wv | wo
            tokN_sb = consts.tile([128, EC, 128], F16)      # built on-device

            # ---- DMA issue: SP carries the 4 MiB weight stream (wq
            # halves first, then wk quarter-chunks); ACT carries tokT and
            # the tokN|wv|wo jumbo.  8 transfers = the DMA completion-lane
            # budget, so nothing stalls at issue. ----
            half = HDR + 4 * E
            nc.sync.dma_start(out=wq_sb[:, 0:half], in_=wq_d[:, 0:half])
            nc.sync.dma_start(out=wq_sb[:, half:], in_=wq_d[:, half:])
            wk_r = wk_d.rearrange("p (ec fc e) -> p ec fc e", fc=EC, e=128)
            for g in range(4):
                nc.sync.dma_start(
                    out=wk_sb[:, 2 * g:2 * g + 2, :, :],
                    in_=wk_r[:, 2 * g:2 * g + 2, :, :],
                )

            nc.scalar.dma_start(
                out=tokT_sb, in_=tokT_d.rearrange("p (ec k) -> p ec k", k=L)
            )
            nc.scalar.dma_start(out=jmb_sb, in_=jmb_d[:, :])

            def tokN_v(ec):
                return tokN_sb[:, ec, :]
            def wv_v(ec):
                return jmb_sb[:, ec * 128:(ec + 1) * 128]
            def wo_v(lo, hi):
                return jmb_sb[:, E + lo:E + hi]

            # header views
            tq_v = wq_sb[:, 0:EC]
            bq_v = wq_sb[:, EC:2 * EC]
            bv_v = wq_sb[:, 2 * EC:2 * EC + 1]
            t128_v = wq_sb[:, 2 * EC + 1:HDR]

            # ---- consts + PE-HAM warmup ----
            ones16 = consts.tile([1, 128], F16)
            nc.vector.memset(ones16, 1.0)
            warm16 = consts.tile([128, 128], F16)
            nc.vector.memset(warm16, 0.0)
            wu_ps = psS.tile([128, 1], F32, tag="wu", bufs=1)
            for w in range(150):
                nc.tensor.matmul(wu_ps, warm16, warm16[:, 0:1],
                                 start=(w == 0), stop=(w == 149))

            ident = consts.tile([128, 128], F16)
            from concourse.masks import make_identity
            make_identity(nc, ident)
            for ec in range(EC):
                tp_ps = psS.tile([128, 128], F16, tag="u", name=f"tp{ec}")
                nc.tensor.transpose(tp_ps, tokT_sb[:, ec, 0:128], ident)
                nc.vector.tensor_copy(tokN_sb[:, ec, :], tp_ps)

            def wq_tile(fc, ec):
                off = HDR + (fc * EC + ec) * 128
                return wq_sb[:, off:off + 128]

            # ---- q columns ----
            q_ps = psS.tile([128, EC], F32, tag="q", bufs=1)
            for fc in range(EC):
                for ec in range(EC):
                    nc.tensor.matmul(
                        q_ps[:, fc:fc + 1], wq_tile(fc, ec),
                        tq_v[:, ec:ec + 1],
                        start=(ec == 0), stop=(ec == EC - 1),
                    )
            q16 = sbw.tile([128, EC], F16)
            nc.vector.tensor_add(q16, q_ps, bq_v)

            # ---- u[ec] -> scores, pipelined per arriving wk chunk.
            # u[ec] lands in a rotating 4-bank PSUM set and the u16 cast
            # + scores matmul trail two groups behind, so the in-order PE
            # never stalls on the DVE round trip. ----
            u16 = sbw.tile([128, EC], F16)
            s_ps = psS.tile([1, L], F32, tag="s", bufs=1)
            utags = ["u", "u", "wu", "cx"]
            u_tiles = []

            def emit_score(ec):
                nc.vector.tensor_copy(u16[:, ec:ec + 1], u_tiles[ec])
                nc.tensor.matmul(
                    s_ps, u16[:, ec:ec + 1], tokT_sb[:, ec, :],
                    start=(ec == 0), stop=(ec == EC - 1),
                )

            for ec in range(EC):
                tg = utags[ec % 4]
                u_ps = psS.tile([128, 1], F32, tag=tg,
                                bufs={"u": 2, "wu": 1, "cx": 1}[tg],
                                name=f"u_ps{ec}")
                u_tiles.append(u_ps)
                for fc in range(EC):
                    nc.tensor.matmul(
                        u_ps, wk_sb[:, ec, fc, :], q16[:, fc:fc + 1],
                        start=(fc == 0), stop=(fc == EC - 1),
                    )
                if ec >= 2:
                    emit_score(ec - 2)
            emit_score(EC - 2)
            emit_score(EC - 1)

            # ---- softmax: exp + 1/sum + normalized fp16 attn row ----
            ex_row = sbw.tile([1, L], F32)
            sm = sbw.tile([1, 1], F32)
            nc.scalar.activation(ex_row, s_ps, mybir.ActivationFunctionType.Exp,
                                 bias=0.0, scale=1.0, accum_out=sm)
            rs = sbw.tile([1, 1], F32)
            nc.vector.reciprocal(rs, sm)
            at16 = sbw.tile([1, L], F16)
            nc.vector.tensor_scalar_mul(at16, ex_row, rs)

            # ---- attn row -> column (PE transpose) ----
            atc_ps = psS.tile([128, 1], F16, tag="wu", bufs=1)
            nc.tensor.transpose(atc_ps, at16[0:1, 0:KA], ones16[0:1, 0:1])
            at_colA = sbw.tile([KA, 1], F16)
            nc.vector.tensor_copy(at_colA, atc_ps)

            # ---- t_avg = attn @ tokens: 8 single-group matmuls; the
            # tail token rides as a PE-broadcast scalar times its
            # pre-packed e-major columns ----
            tv_ps = psS.tile([128, EC], F32, tag="q", bufs=1)
            for ec in range(EC):
                nc.tensor.matmul(tv_ps[:, ec:ec + 1], tokN_v(ec),
                                 at_colA, start=True, stop=True)
            tv16 = sbw.tile([128, EC], F16)
            if LT:
                bc_ps = psS.tile([128, 1], F32, tag="u")
                nc.tensor.matmul(bc_ps, ones16, at16[0:1, KA:KA + 1],
                                 start=True, stop=True)
                bc_sb = sbw.tile([128, 1], F32)
                nc.vector.tensor_copy(bc_sb, bc_ps)
                tail16 = sbw.tile([128, EC], F16)
                nc.vector.tensor_scalar_mul(tail16, t128_v, bc_sb)
                nc.vector.tensor_add(tv16, tv_ps, tail16)
            else:
                nc.vector.tensor_copy(tv16, tv_ps)

            # ---- ctx_c = Wv[S_c,:] t_avg + bv[S_c] ----
            ctx_ps = psS.tile([128, 1], F32, tag="cx", bufs=1)
            for ec in range(EC):
                nc.tensor.matmul(
                    ctx_ps, wv_v(ec), tv16[:, ec:ec + 1],
                    start=(ec == 0), stop=(ec == EC - 1),
                )
            ctx16 = sbw.tile([128, 1], F16)
            nc.vector.tensor_add(ctx16, ctx_ps, bv_v)

            # ---- part_c = Wo[:,S_c] ctx_c (+bo): two [1,512] rows,
            # shared-stationary matmul pairs, per-half copy + DMA ----
            o_ps0 = psS.tile([1, 512], F32, tag="o0", bufs=1)
            o_ps1 = psS.tile([1, 512], F32, tag="o1", bufs=1)
            nc.tensor.matmul(o_ps0, ctx16, wo_v(0, 512), start=True, stop=True)
            nc.tensor.matmul(o_ps1, ctx16, wo_v(512, 1024),
                             start=True, stop=True)
            out_sb = sbw.tile([1, E], F16)
            nc.vector.tensor_copy(out_sb[0:1, 0:512], o_ps0)
            nc.sync.dma_start(out=out_d[:, 0:512], in_=out_sb[0:1, 0:512])
            nc.scalar.activation(out_sb[0:1, 512:1024], o_ps1,
                                 mybir.ActivationFunctionType.Copy,
                                 bias=0.0, scale=1.0)
            nc.sync.dma_start(out=out_d[:, 512:1024], in_=out_sb[0:1, 512:1024])

    nc.finalize()
    return nc


def _get_nc(L: int, qidx: int):
    key = (L, qidx)
    if key not in _BUILD_CACHE:
        _BUILD_CACHE[key] = _build(L, qidx)
    return _BUILD_CACHE[key]


def _prep_in_maps(matrix, Wq, bq, Wk, bk, Wv, bv, Wo, bo, px, py):
    px = int(px)
    py = int(py)
    rows = np.arange(H)[px - WIN:px + WIN + 1]
    cols = np.arange(W)[py - WIN:py + WIN + 1]
    L = len(cols)
    gr = rows[px]
    qidx = py
    KA = min(128, L)

    tokens = np.asarray(matrix[gr][cols], dtype=np.float32)        # [L, E]
    tokT_p = np.ascontiguousarray(
        tokens.T.astype(np.float16).reshape(EC, 128, L).transpose(1, 0, 2)
    ).reshape(128, EC * L)                                         # [pe, ec*k]
    tokN_p = np.ascontiguousarray(
        tokens[:KA].astype(np.float16)
    ).reshape(KA, EC * 128)                                        # [pk, ec*e]
    tq_c = tokens[qidx].astype(np.float16).reshape(EC, 128).T      # [128, ec]
    if L > KA:
        t128_c = tokens[KA].astype(np.float16).reshape(EC, 128).T  # [128, ec]
    else:
        t128_c = np.zeros((128, EC), np.float16)
    bq_c = np.asarray(bq, np.float32).astype(np.float16).reshape(EC, 128).T

    wq_core = (
        Wq.T.astype(np.float16).reshape(EC, 128, EC, 128)
        .transpose(1, 2, 0, 3).reshape(128, EC * E)
    )                                                              # [pe, fc,ec,f]
    wk_p = np.ascontiguousarray(
        (np.asarray(Wk, np.float32) * SCALE).astype(np.float16)
        .reshape(EC, 128, EC, 128).transpose(1, 2, 0, 3)
    ).reshape(128, EC * E)                                         # [pf, ec,fc,e]

    Wv32 = np.asarray(Wv, np.float32)
    Wo32 = np.asarray(Wo, np.float32)
    bv16 = np.asarray(bv, np.float32).astype(np.float16)

    in_maps = []
    for c in range(N_CORES):
        fc = slice(128 * c, 128 * (c + 1))
        hdr = np.concatenate(
            [tq_c, bq_c, bv16[fc][:, None], t128_c], axis=1
        )                                                          # [128, 25]
        wq_ext = np.ascontiguousarray(
            np.concatenate([hdr, wq_core], axis=1)
        )                                                          # [128, 25+8192]
        wv_p = np.ascontiguousarray(
            Wv32[fc, :].T.astype(np.float16).reshape(EC, 128, 128)
            .transpose(1, 0, 2)
        ).reshape(128, E)                                          # [pe, ec*j]
        wo_p = np.ascontiguousarray(Wo32[:, fc].T.astype(np.float16))
        jmb = np.ascontiguousarray(
            np.concatenate([wv_p, wo_p], axis=1)
        )                                                          # [128, 3072]
        in_maps.append({
            "wqx": wq_ext,
            "wk": wk_p,
            "tokT": tokT_p,
            "jmb": jmb,
        })
    return in_maps, L, qidx


def kernel(matrix, Wq, bq, Wk, bk, Wv, bv, Wo, bo, px, py, _trace=False, **_kw):
    in_maps, L, qidx = _prep_in_maps(
        matrix, Wq, bq, Wk, bk, Wv, bv, Wo, bo, px, py
    )
    nc = _get_nc(L, qidx)
    res = run_bass_kernel_spmd(
        nc, in_maps, core_ids=list(range(N_CORES)), trace=_trace
    )
    out = np.sum(
        [res.results[c]["out"][0] for c in range(N_CORES)], axis=0,
        dtype=np.float32,
    ) + np.asarray(bo, np.float32)
    if _trace:
        return out.astype(np.float32), res
    return out.astype(np.float32)
